# revision 1
# baseline (speedup 1.0000x reference)
"""Trainium2 Bass kernel for a batched HGNN layer.

Per batch b (N=4096 nodes, E=2048 hyperedges, C=128 channels):
    De = sum_n H[n,e] + eps                 (hyperedge degrees)
    Dv = sum_e H[n,e] + eps                 (node degrees)
    s  = 1/sqrt(Dv)
    out = ((H @ ((H^T @ (x * s)) / De)) * s) @ W^T + b

Sharding: batch dim B=8, one batch per NeuronCore (data parallel, no
cross-core communication). Inside a core:

  pass 1 (stream H once from HBM, casting fp32->bf16 in the DMA):
    - Dv row-sums on DVE, s on DVE+ACT
    - out2T[c,e] = (x*s)^T @ H accumulated in PSUM (PE)
    - H^T tiles built with PE transposes, cached in SBUF (16 MB bf16)
  interlude:
    - De from free-dim reduces over the cached H^T (DVE)
    - out3[e,c] = transpose(out2T) * (1/De)  (PE transpose + ACT scale)
  pass 2 (H^T streamed from SBUF, no HBM traffic):
    - out4T[c,n] = out3^T @ H^T  (PE, bf16)
    - out[n,co] = (out4T_tile^T @ W^T) * s + b  (PE fp32 + DVE epilogue)

HBM traffic per core = 32 MB (H) + 2 MB (x) + 2 MB (out) ~= 36 MB,
i.e. the memory roofline for this problem.
"""
import os
import sys

import numpy as np

for _p in ("/opt/trn_rl_repo", "/root/.axon_site/_ro/trn_rl_repo"):
    if os.path.isdir(_p) and _p not in sys.path:
        sys.path.append(_p)

B, N, E, C = 8, 4096, 2048, 128
NCHUNKS = N // 128          # 32 row chunks in pass 1
ETILES = E // 128           # 16 hyperedge tiles
NBLKS = N // 512            # 8 column blocks in pass 2
EPS = 1e-6

_CACHE = {}


def _build_nc():
    from contextlib import ExitStack

    import concourse.tile as tile
    from concourse import bacc, mybir

    F32 = mybir.dt.float32
    BF16 = mybir.dt.bfloat16
    X = mybir.AxisListType.X

    nc = bacc.Bacc("TRN2", target_bir_lowering=False, debug=False)

    H_d = nc.dram_tensor("H", [N, E], F32, kind="ExternalInput")
    x_d = nc.dram_tensor("x", [N, C], F32, kind="ExternalInput")
    W_d = nc.dram_tensor("W", [C, C], F32, kind="ExternalInput")
    b_d = nc.dram_tensor("b", [1, C], F32, kind="ExternalInput")
    out_d = nc.dram_tensor("out", [N, C], F32, kind="ExternalOutput")

    H_ap, x_ap, out_ap = H_d.ap(), x_d.ap(), out_d.ap()

    with tile.TileContext(nc) as tc:
        with ExitStack() as ctx:
            const = ctx.enter_context(tc.tile_pool(name="const", bufs=1))
            hpool = ctx.enter_context(tc.tile_pool(name="hp", bufs=3))
            xpool = ctx.enter_context(tc.tile_pool(name="xp", bufs=2))
            spool = ctx.enter_context(tc.tile_pool(name="sp", bufs=2))
            opool = ctx.enter_context(tc.tile_pool(name="op", bufs=3))
            psA = ctx.enter_context(tc.tile_pool(name="psA", bufs=1, space="PSUM"))
            psT = ctx.enter_context(tc.tile_pool(name="psT", bufs=4, space="PSUM"))

            # --- constants -------------------------------------------------
            ident16 = const.tile([128, 128], BF16)
            nc.vector.memset(ident16[:], 1.0)
            nc.gpsimd.affine_select(
                ident16[:], ident16[:], pattern=[[-1, 128]], base=0,
                channel_multiplier=1, compare_op=mybir.AluOpType.is_equal,
                fill=0.0,
            )
            identf = const.tile([128, 128], F32)
            nc.vector.memset(identf[:], 1.0)
            nc.gpsimd.affine_select(
                identf[:], identf[:], pattern=[[-1, 128]], base=0,
                channel_multiplier=1, compare_op=mybir.AluOpType.is_equal,
                fill=0.0,
            )

            w_sb = const.tile([128, 128], F32)
            nc.sync.dma_start(w_sb[:], W_d.ap())
            wt_ps = psT.tile([128, 128], F32, tag="stg")
            nc.tensor.transpose(wt_ps[:], w_sb[:], identf[:])
            wt_sb = const.tile([128, 128], F32)          # W^T: [c_in, c_out]
            nc.scalar.copy(wt_sb[:], wt_ps[:])

            b_sb = const.tile([1, 128], F32)
            nc.sync.dma_start(b_sb[:], b_d.ap())
            ones1 = const.tile([1, 128], F32)
            nc.vector.memset(ones1[:], 1.0)
            bb_ps = psT.tile([128, 128], F32, tag="stg")
            nc.tensor.matmul(bb_ps[:], ones1[:], b_sb[:], start=True, stop=True)
            b_bcast = const.tile([128, 128], F32)        # b replicated per row
            nc.scalar.copy(b_bcast[:], bb_ps[:])

            # --- persistent state ------------------------------------------
            HT = const.tile([128, ETILES * N], BF16)     # H^T cache, 128 KB/part
            out3 = const.tile([128, ETILES * 128], BF16)  # (H^T xs)/De, [e, c]
            Isd = const.tile([128, NCHUNKS], F32)        # 1/sqrt(Dv), col per chunk
            DeP = const.tile([128, ETILES], F32)
            RecDe = const.tile([128, ETILES], F32)

            out2T_ps = psA.tile([128, E], F32)           # 4 PSUM banks

            HT3 = HT[:].rearrange("p (j n) -> p j n", j=ETILES)

            # --- pass 1: stream H from HBM ---------------------------------
            for i in range(NCHUNKS):
                h16 = hpool.tile([128, E], BF16)
                nc.gpsimd.dma_start(h16[:], H_ap[i * 128:(i + 1) * 128, :])
                x_t = xpool.tile([128, C], F32, tag="x")
                nc.sync.dma_start(x_t[:], x_ap[i * 128:(i + 1) * 128, :])

                dv = spool.tile([128, 1], F32, tag="dv")
                nc.vector.reduce_sum(dv[:], h16[:], axis=X)
                rec = spool.tile([128, 1], F32, tag="rec")
                nc.vector.tensor_scalar_add(rec[:], dv[:], EPS)
                nc.vector.reciprocal(rec[:], rec[:])
                nc.scalar.sqrt(Isd[:, i:i + 1], rec[:])

                xs16 = xpool.tile([128, C], BF16, tag="xs")
                nc.scalar.mul(xs16[:], x_t[:], Isd[:, i:i + 1])

                for s in range(4):
                    nc.tensor.matmul(
                        out2T_ps[:, s * 512:(s + 1) * 512],
                        xs16[:], h16[:, s * 512:(s + 1) * 512],
                        start=(i == 0), stop=(i == NCHUNKS - 1),
                    )

                for g in range(2):
                    stg = psT.tile([128, 1024], BF16, tag="stg")
                    for k in range(8):
                        j = g * 8 + k
                        nc.tensor.transpose(
                            stg[:, k * 128:(k + 1) * 128],
                            h16[:, j * 128:(j + 1) * 128], ident16[:],
                        )
                    dest = HT3[:, g * 8:(g + 1) * 8, i * 128:(i + 1) * 128]
                    src = stg[:].rearrange("p (k n) -> p k n", k=8)
                    if g == 0:
                        nc.scalar.copy(dest, src)
                    else:
                        nc.vector.tensor_copy(dest, src)

            # --- interlude: De, out3 ---------------------------------------
            for j in range(ETILES):
                nc.vector.reduce_sum(
                    DeP[:, j:j + 1], HT[:, j * N:(j + 1) * N], axis=X
                )
            nc.vector.tensor_scalar_add(RecDe[:], DeP[:], EPS)
            nc.vector.reciprocal(RecDe[:], RecDe[:])

            o2_sb = const.tile([128, E], F32)
            nc.scalar.copy(o2_sb[:], out2T_ps[:])
            for j in range(ETILES):
                t2 = psT.tile([128, 128], F32, tag="stg")
                nc.tensor.transpose(
                    t2[:], o2_sb[:, j * 128:(j + 1) * 128], identf[:]
                )
                nc.scalar.mul(
                    out3[:, j * 128:(j + 1) * 128], t2[:], RecDe[:, j:j + 1]
                )

            # --- pass 2: H^T from SBUF -------------------------------------
            for blk in range(NBLKS):
                o4 = psT.tile([128, 512], F32, tag="stg")
                for j in range(ETILES):
                    nc.tensor.matmul(
                        o4[:],
                        out3[:, j * 128:(j + 1) * 128],
                        HT[:, j * N + blk * 512:j * N + (blk + 1) * 512],
                        start=(j == 0), stop=(j == ETILES - 1),
                    )
                o4sb = opool.tile([128, 512], F32, tag="o4")
                nc.scalar.copy(o4sb[:], o4[:])
                for t in range(4):
                    idx = blk * 4 + t
                    lp = psT.tile([128, 128], F32, tag="stg")
                    nc.tensor.matmul(
                        lp[:], o4sb[:, t * 128:(t + 1) * 128], wt_sb[:],
                        start=True, stop=True,
                    )
                    o = opool.tile([128, 128], F32, tag="o")
                    nc.vector.tensor_scalar_mul(o[:], lp[:], Isd[:, idx:idx + 1])
                    nc.vector.tensor_add(o[:], o[:], b_bcast[:])
                    nc.sync.dma_start(
                        out_ap[idx * 128:(idx + 1) * 128, :], o[:]
                    )

    nc.compile()
    return nc


def _get_nc():
    if "nc" not in _CACHE:
        _CACHE["nc"] = _build_nc()
    return _CACHE["nc"]


def kernel(x, H, W, b):
    from concourse.bass_utils import run_bass_kernel_spmd

    nc = _get_nc()
    x = np.ascontiguousarray(x, dtype=np.float32)
    H = np.ascontiguousarray(H, dtype=np.float32)
    W = np.ascontiguousarray(W, dtype=np.float32)
    b2 = np.ascontiguousarray(b, dtype=np.float32).reshape(1, C)
    in_maps = [
        {"x": x[c], "H": H[c], "W": W, "b": b2} for c in range(B)
    ]
    res = run_bass_kernel_spmd(nc, in_maps, core_ids=list(range(B)))
    return np.stack([res.results[c]["out"] for c in range(B)], axis=0)


# revision 3
# speedup vs baseline: 1.3573x; 1.3573x over previous
"""Trainium2 Bass kernel for a batched HGNN layer.

Per batch b (N=4096 nodes, E=2048 hyperedges, C=128 channels):
    De = sum_n H[n,e] + eps                 (hyperedge degrees)
    Dv = sum_e H[n,e] + eps                 (node degrees)
    s  = 1/sqrt(Dv)
    out = ((H @ ((H^T @ (x * s)) / De)) * s) @ W^T + b

Sharding: batch dim B=8, one batch per NeuronCore (data parallel, no
cross-core communication). Inside a core:

  pass 1 (streams H once from HBM, in 8 superchunks of 512 rows):
    - fp32->bf16 cast on ACT/DVE with fused row-sum (Dv) via accum_out
    - out2T[c,e] = (x*s)^T @ H accumulated in PSUM (PE, bf16)
    - H^T built with PE transposes (j-major staging), cached in SBUF
      (16 MB bf16); the PSUM->SBUF staging copies carry fused column-sum
      partials (De) via accum_out
  interlude:
    - out3[e,c] = transpose(out2T) * (1/De)  (PE transpose + ACT scale)
  pass 2 (H^T streamed from SBUF, no HBM traffic):
    - out4T[c,n] = out3^T @ H^T  (PE, bf16)
    - out[n,co] = (out4T_tile^T @ W^T) * s + b  (PE fp32 + DVE epilogue)

HBM traffic per core = 32 MB (H) + 2 MB (x) + 2 MB (out) ~= 36 MB,
i.e. the memory roofline for this problem.
"""
import os
import sys

import numpy as np

for _p in ("/opt/trn_rl_repo", "/root/.axon_site/_ro/trn_rl_repo"):
    if os.path.isdir(_p) and _p not in sys.path:
        sys.path.append(_p)

B, N, E, C = 8, 4096, 2048, 128
SC = 4                      # subchunks (128 rows) per superchunk
NSUPER = N // (128 * SC)    # 8 superchunks in pass 1
NCHUNKS = N // 128          # 32 row chunks
ETILES = E // 128           # 16 hyperedge tiles
NBLKS = N // 512            # 8 column blocks in pass 2
EPS = 1e-6

_CACHE = {}


def _build_nc():
    from contextlib import ExitStack

    import concourse.tile as tile
    from concourse import bacc, mybir

    F32 = mybir.dt.float32
    BF16 = mybir.dt.bfloat16
    X = mybir.AxisListType.X
    ADD = mybir.AluOpType.add
    COPY = mybir.ActivationFunctionType.Copy

    nc = bacc.Bacc("TRN2", target_bir_lowering=False, debug=False)

    H_d = nc.dram_tensor("H", [N, E], F32, kind="ExternalInput")
    x_d = nc.dram_tensor("x", [N, C], F32, kind="ExternalInput")
    W_d = nc.dram_tensor("W", [C, C], F32, kind="ExternalInput")
    b_d = nc.dram_tensor("b", [1, C], F32, kind="ExternalInput")
    out_d = nc.dram_tensor("out", [N, C], F32, kind="ExternalOutput")

    H_ap, x_ap, out_ap = H_d.ap(), x_d.ap(), out_d.ap()

    with tile.TileContext(nc) as tc:
        with ExitStack() as ctx:
            const = ctx.enter_context(tc.tile_pool(name="const", bufs=1))
            h32p = ctx.enter_context(tc.tile_pool(name="h32", bufs=2))
            h16p = ctx.enter_context(tc.tile_pool(name="h16", bufs=2))
            xpool = ctx.enter_context(tc.tile_pool(name="xp", bufs=2))
            spool = ctx.enter_context(tc.tile_pool(name="sp", bufs=2))
            opool = ctx.enter_context(tc.tile_pool(name="op", bufs=2))
            psA = ctx.enter_context(tc.tile_pool(name="psA", bufs=1, space="PSUM"))
            psT = ctx.enter_context(tc.tile_pool(name="psT", bufs=4, space="PSUM"))

            # --- constants -------------------------------------------------
            ident16 = const.tile([128, 128], BF16)
            nc.vector.memset(ident16[:], 1.0)
            nc.gpsimd.affine_select(
                ident16[:], ident16[:], pattern=[[-1, 128]], base=0,
                channel_multiplier=1, compare_op=mybir.AluOpType.is_equal,
                fill=0.0,
            )
            identf = const.tile([128, 128], F32)
            nc.vector.memset(identf[:], 1.0)
            nc.gpsimd.affine_select(
                identf[:], identf[:], pattern=[[-1, 128]], base=0,
                channel_multiplier=1, compare_op=mybir.AluOpType.is_equal,
                fill=0.0,
            )

            w_sb = const.tile([128, 128], F32)
            nc.sync.dma_start(w_sb[:], W_d.ap())
            wt_ps = psT.tile([128, 128], F32, tag="stg")
            nc.tensor.transpose(wt_ps[:], w_sb[:], identf[:])
            wt_sb = const.tile([128, 128], F32)          # W^T: [c_in, c_out]
            nc.scalar.copy(wt_sb[:], wt_ps[:])

            b_sb = const.tile([1, 128], F32)
            nc.sync.dma_start(b_sb[:], b_d.ap())
            ones1 = const.tile([1, 128], F32)
            nc.vector.memset(ones1[:], 1.0)
            bb_ps = psT.tile([128, 128], F32, tag="stg")
            nc.tensor.matmul(bb_ps[:], ones1[:], b_sb[:], start=True, stop=True)
            b_bcast = const.tile([128, 128], F32)        # b replicated per row
            nc.scalar.copy(b_bcast[:], bb_ps[:])

            # --- persistent state ------------------------------------------
            HT = const.tile([128, ETILES * N], BF16)     # H^T cache, 128 KB/part
            out3 = const.tile([128, ETILES * 128], BF16)  # (H^T xs)/De, [e, c]
            Isd = const.tile([128, NCHUNKS], F32)        # 1/sqrt(Dv)
            DvRaw = const.tile([128, NCHUNKS], F32)
            DeP2 = const.tile([128, ETILES * NSUPER], F32)  # De partials
            DeP = const.tile([128, ETILES], F32)
            RecDe = const.tile([128, ETILES], F32)

            out2T_ps = psA.tile([128, E], F32)           # 4 PSUM banks

            HT3 = HT[:].rearrange("p (j n) -> p j n", j=ETILES)

            # --- pass 1: stream H from HBM ---------------------------------
            for i in range(NSUPER):
                h16s = h16p.tile([128, SC, E], BF16)
                for t in range(SC):
                    ci = i * SC + t
                    h32 = h32p.tile([128, E], F32)
                    nc.sync.dma_start(
                        h32[:], H_ap[ci * 128:(ci + 1) * 128, :]
                    )
                    # cast fp32->bf16 with fused Dv row-sum (fp32 accum)
                    if t % 2 == 0:
                        nc.scalar.activation(
                            h16s[:, t, :], h32[:], COPY,
                            accum_out=DvRaw[:, ci:ci + 1],
                        )
                    else:
                        nc.vector.tensor_scalar(
                            h16s[:, t, :], h32[:], 0.0, None, ADD, ADD,
                            accum_out=DvRaw[:, ci:ci + 1],
                        )

                # 1/sqrt(Dv+eps) for the 4 chunks at once
                rec = spool.tile([128, SC], F32, tag="rec")
                nc.vector.tensor_scalar_add(
                    rec[:], DvRaw[:, i * SC:(i + 1) * SC], EPS
                )
                nc.vector.reciprocal(rec[:], rec[:])
                nc.scalar.sqrt(Isd[:, i * SC:(i + 1) * SC], rec[:])

                x_t = xpool.tile([128, SC, C], F32, tag="x")
                nc.sync.dma_start(
                    x_t[:],
                    x_ap[i * SC * 128:(i + 1) * SC * 128, :].rearrange(
                        "(t p) c -> p t c", p=128
                    ),
                )
                xs16 = xpool.tile([128, SC, C], BF16, tag="xs")
                for t in range(SC):
                    ci = i * SC + t
                    nc.scalar.mul(
                        xs16[:, t, :], x_t[:, t, :], Isd[:, ci:ci + 1]
                    )

                # out2T += xs^T @ H   (bf16, PSUM fp32 accumulate)
                for t in range(SC):
                    for s in range(4):
                        nc.tensor.matmul(
                            out2T_ps[:, s * 512:(s + 1) * 512],
                            xs16[:, t, :],
                            h16s[:, t, s * 512:(s + 1) * 512],
                            start=(i == 0 and t == 0),
                            stop=(i == NSUPER - 1 and t == SC - 1),
                        )

                # H^T tiles: PE transpose -> PSUM staging -> SBUF cache,
                # with fused De column-sum partials on the copies
                for j in range(ETILES):
                    stg = psT.tile([128, SC * 128], BF16, tag="stg")
                    for t in range(SC):
                        nc.tensor.transpose(
                            stg[:, t * 128:(t + 1) * 128],
                            h16s[:, t, j * 128:(j + 1) * 128],
                            ident16[:],
                        )
                    dcol = j * NSUPER + i
                    dest = HT3[:, j, i * SC * 128:(i + 1) * SC * 128]
                    if j % 2 == 0:
                        nc.scalar.activation(
                            dest, stg[:], COPY,
                            accum_out=DeP2[:, dcol:dcol + 1],
                        )
                    else:
                        nc.vector.tensor_scalar(
                            dest, stg[:], 0.0, None, ADD, ADD,
                            accum_out=DeP2[:, dcol:dcol + 1],
                        )

            # --- interlude: De, out3 ---------------------------------------
            nc.vector.reduce_sum(
                DeP[:],
                DeP2[:].rearrange("p (j i) -> p j i", j=ETILES),
                axis=X,
            )
            nc.vector.tensor_scalar_add(RecDe[:], DeP[:], EPS)
            nc.vector.reciprocal(RecDe[:], RecDe[:])

            o2_sb = const.tile([128, E], F32)
            nc.scalar.copy(o2_sb[:], out2T_ps[:])
            for j in range(ETILES):
                t2 = psT.tile([128, 128], F32, tag="stg")
                nc.tensor.transpose(
                    t2[:], o2_sb[:, j * 128:(j + 1) * 128], identf[:]
                )
                nc.scalar.mul(
                    out3[:, j * 128:(j + 1) * 128], t2[:], RecDe[:, j:j + 1]
                )

            # --- pass 2: H^T from SBUF -------------------------------------
            for blk in range(NBLKS):
                o4 = psT.tile([128, 512], F32, tag="stg")
                for j in range(ETILES):
                    nc.tensor.matmul(
                        o4[:],
                        out3[:, j * 128:(j + 1) * 128],
                        HT[:, j * N + blk * 512:j * N + (blk + 1) * 512],
                        start=(j == 0), stop=(j == ETILES - 1),
                    )
                o4sb = opool.tile([128, 512], F32, tag="o4")
                nc.scalar.copy(o4sb[:], o4[:])
                for t in range(4):
                    idx = blk * 4 + t
                    lp = psT.tile([128, 128], F32, tag="stg")
                    nc.tensor.matmul(
                        lp[:], o4sb[:, t * 128:(t + 1) * 128], wt_sb[:],
                        start=True, stop=True,
                    )
                    o = opool.tile([128, 128], F32, tag="o")
                    nc.vector.tensor_scalar_mul(o[:], lp[:], Isd[:, idx:idx + 1])
                    nc.vector.tensor_add(o[:], o[:], b_bcast[:])
                    nc.sync.dma_start(
                        out_ap[idx * 128:(idx + 1) * 128, :], o[:]
                    )

    nc.compile()
    return nc


def _get_nc():
    if "nc" not in _CACHE:
        _CACHE["nc"] = _build_nc()
    return _CACHE["nc"]


def kernel(x, H, W, b):
    from concourse.bass_utils import run_bass_kernel_spmd

    nc = _get_nc()
    x = np.ascontiguousarray(x, dtype=np.float32)
    H = np.ascontiguousarray(H, dtype=np.float32)
    W = np.ascontiguousarray(W, dtype=np.float32)
    b2 = np.ascontiguousarray(b, dtype=np.float32).reshape(1, C)
    in_maps = [
        {"x": x[c], "H": H[c], "W": W, "b": b2} for c in range(B)
    ]
    res = run_bass_kernel_spmd(nc, in_maps, core_ids=list(range(B)))
    return np.stack([res.results[c]["out"] for c in range(B)], axis=0)
